# revision 33
# baseline (speedup 1.0000x reference)
"""AttentionBlock3D kernel for 8 Trainium2 NeuronCores (Bass/Tile, SPMD).

Sharding: core c in 0..7 handles batch b = c//4 and query slice
qoff = (c%4)*512 of the N=2048 flattened positions. Each core computes
GroupNorm + full K/V for its batch (replicated across the 4 cores sharing a
batch -> zero cross-core communication), attention for its 512 queries over
all 2048 keys, projection and residual. Host gathers by pure concatenation.

The kernel works in a "transposed" attention layout: scoresT[j, i] (keys on
partitions, queries on the free axis) so that the softmax denominator comes
for free out of the PE via a ones-column appended to V, and no transposes of
the probability matrix are needed (softmax needs no max-subtraction: scores
are O(1) for this block). The [N, N] relative-position bias enters as
exp(bias) (host-gathered from rel_emb, bf16): et = exp(qk/sqrt(d)) * exp(b)
multiplied on the vector engine in 2x bf16 mode. Softmax denominators are
repacked [1,512]->[128,4] by DMA so one cheap reciprocal serves each head
pair, then broadcast across partitions on GpSimd. Matmul operands are bf16
(the GroupNorm statistics path stays float32/float32r); accumulation is fp32.
K projections for later head-pairs and the V projection are interleaved into
the attention stream to keep the PE busy while ACT/DVE chew exp/multiplies.

Per-core inputs are rotated along the position axis by -qoff so that one
SPMD program (query slice = columns 0:512) serves all cores; GroupNorm and
softmax are permutation-invariant so results are unaffected.
"""
import sys

sys.path.insert(0, "/opt/trn_rl_repo")

from contextlib import ExitStack

import numpy as np

import concourse.bacc as bacc
import concourse.mybir as mybir
import concourse.tile as tile
from concourse.bass_utils import run_bass_kernel_spmd

B, C, D, H, W = 2, 512, 8, 16, 16
N = D * H * W  # 2048
HEADS, HD = 8, 64
GROUPS = 8
NUM_BUCKETS = 32
MAX_DIST = 128.0
EPS = 1e-5
NCORES = 8
NQ = N // 4  # 512 queries per core
F32 = mybir.dt.float32
F32R = mybir.dt.float32r
BF16 = mybir.dt.bfloat16

_CACHE = {}


def _build():
    nc = bacc.Bacc(
        "TRN2", target_bir_lowering=False, debug=False, num_devices=NCORES
    )
    AF = mybir.ActivationFunctionType
    OP = mybir.AluOpType

    x_d = nc.dram_tensor("x", [C, N], BF16, kind="ExternalInput").ap()
    xres_d = nc.dram_tensor("xres", [C, NQ], F32, kind="ExternalInput").ap()
    qkvwT_d = nc.dram_tensor("qkvwT", [C, 3 * C], BF16, kind="ExternalInput").ap()
    projwT_d = nc.dram_tensor("projwT", [C, C], BF16, kind="ExternalInput").ap()
    bias_d = nc.dram_tensor("expbT", [N, NQ], BF16, kind="ExternalInput").ap()
    gnw_d = nc.dram_tensor("gnw", [C], F32, kind="ExternalInput").ap()
    gnb_d = nc.dram_tensor("gnb", [C], F32, kind="ExternalInput").ap()
    qkvb_d = nc.dram_tensor("qkvb", [3 * C], F32, kind="ExternalInput").ap()
    projb_d = nc.dram_tensor("projb", [C], F32, kind="ExternalInput").ap()
    gsel_d = nc.dram_tensor("gsel", [C, GROUPS], F32R, kind="ExternalInput").ap()
    gselT_d = nc.dram_tensor("gselT", [GROUPS, C], F32R, kind="ExternalInput").ap()
    ones8_d = nc.dram_tensor("ones8", [128, HEADS], BF16, kind="ExternalInput").ap()
    out_d = nc.dram_tensor("out", [C, NQ], F32, kind="ExternalOutput").ap()

    with tile.TileContext(nc) as tc, ExitStack() as ctx:
        mb = ctx.enter_context(tc.tile_pool(name="mb", bufs=15))
        vg = ctx.enter_context(tc.tile_pool(name="vg", bufs=1))
        ex = ctx.enter_context(tc.tile_pool(name="ex", bufs=1))
        sm = ctx.enter_context(tc.tile_pool(name="sm", bufs=1))
        one = ctx.enter_context(tc.tile_pool(name="one", bufs=1))
        ps2 = ctx.enter_context(tc.tile_pool(name="ps2", bufs=1, space="PSUM"))
        ps1 = ctx.enter_context(tc.tile_pool(name="ps1", bufs=1, space="PSUM"))

        # ---- x load --------------------------------------------------
        xh = []
        for t in range(4):
            xt = mb.tile([128, N], BF16, tag="mb", name=f"xh{t}")
            nc.sync.dma_start(out=xt, in_=x_d[128 * t : 128 * (t + 1), :])
            xh.append(xt)

        # pre-warm ACT table sets off the critical path: Sqrt now (during
        # the x DMA), Exp after GroupNorm (during the qkv phase)
        warm = one.tile([1, 1], F32)
        nc.vector.memset(warm, 1.0)
        warm_eps = one.tile([1, 1], F32)
        nc.vector.memset(warm_eps, 0.0)
        nc.scalar.activation(
            out=warm, in_=warm, func=AF.Sqrt, bias=warm_eps, scale=1.0
        )
        gsel = one.tile([128, 4, GROUPS], F32R)
        nc.sync.dma_start(out=gsel, in_=gsel_d.rearrange("(a p) g -> p a g", p=128))
        gselT = one.tile([GROUPS, C], F32R)
        nc.sync.dma_start(out=gselT, in_=gselT_d)
        ones8 = one.tile([128, HEADS], BF16)
        nc.sync.dma_start(out=ones8, in_=ones8_d)
        gnw = one.tile([128, 4], F32)
        nc.sync.dma_start(out=gnw, in_=gnw_d.rearrange("(a p) -> p a", p=128))
        gnb = one.tile([128, 4], F32)
        nc.sync.dma_start(out=gnb, in_=gnb_d.rearrange("(a p) -> p a", p=128))
        qkvb = one.tile([128, 12], F32)  # col 4*s+t = channels [s*512+128t..+128)
        nc.sync.dma_start(
            out=qkvb, in_=qkvb_d.rearrange("(s a p) -> p (s a)", p=128, s=3)
        )
        projb = one.tile([128, 4], F32)
        nc.sync.dma_start(out=projb, in_=projb_d.rearrange("(a p) -> p a", p=128))

        # ---- constants / weights -------------------------------------
        # qkv weight, split per projection: w[s][:, ct, o] for s in (q, k, v)
        wqkv = []
        for s in range(3):
            ws = mb.tile([128, 4, C], BF16, tag="mb", name=f"w{'qkv'[s]}")
            nc.sync.dma_start(
                out=ws,
                in_=qkvwT_d[:, C * s : C * (s + 1)].rearrange(
                    "(a p) o -> p a o", p=128
                ),
            )
            wqkv.append(ws)
        wq, wk, wv = wqkv
        # ---- GroupNorm ----------------------------------------------
        # Per-channel (mean, E[x^2]) via bn_stats, then group-reduce across
        # partitions with a one-hot matmul, rsqrt, broadcast back, affine.
        ps_g = ps2.tile([128, 512], F32, tag="ps_s", bufs=3, name="ps_g")
        ms_list = []
        for t in range(4):
            stats = sm.tile([128, 4, 6], F32, tag="stats", bufs=4, name=f"st{t}")
            for sg in range(4):
                nc.vector.bn_stats(
                    out=stats[:, sg, :], in_=xh[t][:, 512 * sg : 512 * (sg + 1)]
                )
            mv = sm.tile([128, 2], F32, tag="mv", bufs=2, name=f"mv{t}")
            nc.vector.bn_aggr(out=mv, in_=stats)
            ms = sm.tile([128, 2], F32R, tag="ms", bufs=4, name=f"ms{t}")
            nc.vector.tensor_copy(out=ms[:, 0:1], in_=mv[:, 0:1])
            nc.vector.tensor_tensor(
                out=ms[:, 1:2], in0=mv[:, 0:1], in1=mv[:, 0:1], op=OP.mult
            )
            nc.vector.tensor_tensor(
                out=ms[:, 1:2], in0=ms[:, 1:2], in1=mv[:, 1:2], op=OP.add
            )
            ms_list.append(ms)
            nc.tensor.matmul(
                ps_g[0:GROUPS, 0:2],
                lhsT=gsel[:, t, :],
                rhs=ms,
                start=(t == 0),
                stop=(t == 3),
            )
        gsc = sm.tile([GROUPS, 2], F32, name="gsc")  # (mu_g, E[x^2]_g)
        nc.vector.tensor_scalar_mul(
            out=gsc, in0=ps_g[0:GROUPS, 0:2], scalar1=1.0 / 64.0
        )
        var = sm.tile([GROUPS, 1], F32, name="var")
        nc.vector.tensor_tensor(
            out=var, in0=gsc[:, 0:1], in1=gsc[:, 0:1], op=OP.mult
        )
        nc.vector.tensor_tensor(
            out=var, in0=gsc[:, 1:2], in1=var, op=OP.subtract
        )
        eps_t = sm.tile([GROUPS, 1], F32, name="eps_t")
        nc.vector.memset(eps_t, EPS)
        sd = sm.tile([GROUPS, 1], F32, name="sd")
        nc.scalar.activation(out=sd, in_=var, func=AF.Sqrt, bias=eps_t, scale=1.0)
        rstd = sm.tile([GROUPS, 1], F32, name="rstd")
        nc.vector.reciprocal(out=rstd, in_=sd)
        grhs = sm.tile([GROUPS, 2], F32R, name="grhs")
        nc.vector.tensor_copy(out=grhs[:, 0:1], in_=rstd)
        nc.vector.tensor_copy(out=grhs[:, 1:2], in_=gsc[:, 0:1])

        h_r = []
        for t in range(4):
            ps_b = ps2.tile([128, 512], F32, tag="ps_s", bufs=3, name=f"ps_b{t}")
            nc.tensor.matmul(
                ps_b[:, 0:2],
                lhsT=gselT[:, 128 * t : 128 * (t + 1)],
                rhs=grhs,
                start=True,
                stop=True,
            )
            a_c = sm.tile([128, 1], F32, tag="a_c", bufs=4, name=f"a_c{t}")
            nc.vector.tensor_tensor(
                out=a_c, in0=gnw[:, t : t + 1], in1=ps_b[:, 0:1], op=OP.mult
            )
            b_c = sm.tile([128, 1], F32, tag="b_c", bufs=4, name=f"b_c{t}")
            nc.vector.tensor_tensor(
                out=b_c, in0=ps_b[:, 1:2], in1=a_c, op=OP.mult
            )
            nc.vector.tensor_tensor(
                out=b_c, in0=gnb[:, t : t + 1], in1=b_c, op=OP.subtract
            )
            ht = mb.tile([128, N], BF16, tag="mb", name=f"h{t}")
            nc.vector.tensor_scalar(
                out=ht,
                in0=xh[t],
                scalar1=a_c,
                scalar2=b_c,
                op0=OP.mult,
                op1=OP.add,
            )
            h_r.append(ht)

        warm2 = one.tile([1, 1], F32)
        nc.vector.memset(warm2, 1.0)
        nc.scalar.activation(out=warm2, in_=warm2, func=AF.Exp, scale=1.0)
        # bias tiles (4 x [128, 4, 512]); tile q holds j-blocks 4q..4q+3
        bias_t = []
        for q in range(4):
            bt = mb.tile([128, 4, NQ], BF16, tag="mb", name=f"bias{q}")
            nc.sync.dma_start(
                out=bt,
                in_=bias_d[512 * q : 512 * (q + 1), :].rearrange(
                    "(a p) i -> p a i", p=128
                ),
            )
            bias_t.append(bt)

        # ---- qkv projections ----------------------------------------
        # qT: [q-channel, i] for the 512 local queries (columns 0:512)
        qt = mb.tile([128, 4, NQ], BF16, tag="mb", name="qt")
        for op2 in range(2):
            pq = ps2.tile([128, 2, 512], F32, tag="ps_s", bufs=3, name=f"pq{op2}")
            for oh in range(2):
                for ct in range(4):
                    nc.tensor.matmul(
                        pq[:, oh, :],
                        lhsT=wq[:, ct, 256 * op2 + 128 * oh : 256 * op2 + 128 * oh + 128],
                        rhs=h_r[ct][:, 0:NQ],
                        start=(ct == 0),
                        stop=(ct == 3),
                        skip_group_check=True,
                    )
            # per-partition bias differs across the two o-tiles -> two ops
            for oh in range(2):
                nc.scalar.activation(
                    out=qt[:, 2 * op2 + oh, :],
                    in_=pq[:, oh, :],
                    func=AF.Identity,
                    bias=qkvb[:, 2 * op2 + oh : 2 * op2 + oh + 1],
                    scale=1.0,
                )
        # kT: [k-channel, j] over all 2048 keys. Only the pair-0 tile is
        # emitted up front; later pairs' K projections are interleaved into
        # the attention stream (they fill PE bubbles / pair-boundary tails).
        kt = {}

        def emit_kt(ot):
            ktile = mb.tile([128, N], BF16, tag="mb", name=f"kt{ot}")
            for njp in range(2):
                pk = ps2.tile(
                    [128, 2, 512], F32, tag="ps_s", bufs=3, name=f"pk{ot}{njp}"
                )
                for nh in range(2):
                    for ct in range(4):
                        nc.tensor.matmul(
                            pk[:, nh, :],
                            lhsT=wk[:, ct, 128 * ot : 128 * (ot + 1)],
                            rhs=h_r[ct][
                                :, 1024 * njp + 512 * nh : 1024 * njp + 512 * nh + 512
                            ],
                            start=(ct == 0),
                            stop=(ct == 3),
                            skip_group_check=True,
                        )
                nc.scalar.activation(
                    out=ktile[:, 1024 * njp : 1024 * (njp + 1)],
                    in_=pk.rearrange("p a i -> p (a i)"),
                    func=AF.Identity,
                    bias=qkvb[:, 4 + ot : 4 + ot + 1],
                    scale=1.0,
                )
            kt[ot] = ktile

        for _ot in range(2):
            emit_kt(_ot)
        # V in [j, v-channel] layout, augmented with a ones column per head:
        # vaug[:, nt, h*65+d] = v[j, 64h+d] ; vaug[:, nt, h*65+64] = 1.
        # The v-bias is folded into the projection bias on the host
        # (softmax weights sum to 1, so attn(v+vb) = attn(v) + vb).
        vaug = []
        for q in range(4):
            vt = vg.tile([128, 4, HEADS, 65], BF16, name=f"vaug{q}")
            nc.vector.tensor_copy(
                out=vt[:, :, :, 64:65].squeeze(3),
                in_=ones8.unsqueeze(1).broadcast_to([128, 4, HEADS]),
            )
            vaug.append(vt)
        def emit_v_chunk(ntp):
            pv = ps2.tile([128, 2, 512], F32, tag="ps_s", bufs=3, name=f"pv{ntp}")
            for nh in range(2):
                nt = 2 * ntp + nh
                for ct in range(4):
                    nc.tensor.matmul(
                        pv[:, nh, :],
                        lhsT=h_r[ct][:, 128 * nt : 128 * (nt + 1)],
                        rhs=wv[:, ct, :],
                        start=(ct == 0),
                        stop=(ct == 3),
                        skip_group_check=True,
                    )
            q, jj = (2 * ntp) // 4, (2 * ntp) % 4
            nc.scalar.activation(
                out=vaug[q][:, jj : jj + 2, :, 0:64],
                in_=pv.rearrange("p a (h d) -> p a h d", d=HD),
                func=AF.Copy,
                scale=1.0,
            )

        # ---- attention (head pairs; QK row-packed at base 0/64) -----
        # preload projection weights + residual so proj can start immediately
        projwT = mb.tile([128, 4, C], BF16, tag="mb", name="projwT")
        nc.sync.dma_start(
            out=projwT, in_=projwT_d.rearrange("(a p) o -> p a o", p=128)
        )
        xres = mb.tile([128, 4, NQ], F32, tag="mb", name="xres")
        nc.sync.dma_start(
            out=xres, in_=xres_d.rearrange("(a p) i -> p a i", p=128)
        )
        attnT = mb.tile([128, 4, NQ], BF16, tag="mb", name="attnT")
        for hp in range(4):
            ha, hb = 2 * hp, 2 * hp + 1
            av = {}
            for h, lab in ((ha, "a"), (hb, "b")):
                av[h] = ps1.tile(
                    [128, 512], F32, tag="ps_av", bufs=2, name=f"av{h}"
                )
            pend = []  # delayed AV emission: (h, g, et)
            for g in range(8):
                if hp == 0:
                    emit_v_chunk(g)
                TA = ps2.tile(
                    [128, 2, 512], F32, tag="ps_s", bufs=3, name=f"sa{hp}_{g}"
                )
                TB = ps2.tile(
                    [128, 2, 512], F32, tag="ps_s", bufs=3, name=f"sb{hp}_{g}"
                )
                for jj in range(2):
                    jb = 2 * g + jj
                    js = slice(128 * jb, 128 * (jb + 1))
                    # the two K=64 matmuls run concurrently (row groups 0/64)
                    nc.tensor.matmul(
                        TA[:, jj, :],
                        lhsT=kt[hp][0:64, js],
                        rhs=qt[0:64, hp, :],
                        start=True,
                        stop=True,
                        skip_group_check=True,
                    )
                    nc.tensor.matmul(
                        TB[:, jj, :],
                        lhsT=kt[hp][64:128, js],
                        rhs=qt[64:128, hp, :],
                        start=True,
                        stop=True,
                        skip_group_check=True,
                    )
                new_pend = []
                for h, T in ((ha, TA), (hb, TB)):
                    etr = ex.tile(
                        [128, 2, 512], BF16, tag="etr", bufs=6, name=f"er{h}_{g}"
                    )
                    nc.scalar.activation(out=etr, in_=T, func=AF.Exp, scale=0.125)
                    et = ex.tile(
                        [128, 2, 512], BF16, tag="et", bufs=10, name=f"et{h}_{g}"
                    )
                    jb0 = 2 * g
                    nc.vector.tensor_tensor(
                        out=et,
                        in0=etr,
                        in1=bias_t[jb0 // 4][:, jb0 % 4 : jb0 % 4 + 2, :],
                        op=OP.mult,
                    )
                    new_pend.append((h, g, et))
                pend = pend + new_pend
                depth = 6  # entries = 2 per delayed group
                while len(pend) > depth:
                    h, gp, etp = pend.pop(0)
                    for jj in range(2):
                        jb = 2 * gp + jj
                        nc.tensor.matmul(
                            av[h][0:65, :],
                            lhsT=vaug[jb // 4][:, jb % 4, h, :],
                            rhs=etp[:, jj, :],
                            start=(gp == 0 and jj == 0),
                            stop=(gp == 7 and jj == 1),
                            skip_group_check=True,
                        )
            for h, gp, etp in pend:
                for jj in range(2):
                    jb = 2 * gp + jj
                    nc.tensor.matmul(
                        av[h][0:65, :],
                        lhsT=vaug[jb // 4][:, jb % 4, h, :],
                        rhs=etp[:, jj, :],
                        start=(gp == 0 and jj == 0),
                        stop=(gp == 7 and jj == 1),
                        skip_group_check=True,
                    )
            if hp + 2 < 4:
                emit_kt(hp + 2)
            # normalize: rows 0:63 = unnormalized attn^T, row 64 = denom.
            # Reciprocal free-dim cost is brutal (~6 ns/elem), so repack the
            # two denominator rows into [128, 8] via DMA, reciprocal once,
            # and DMA back for the PE broadcast matmul.
            coll = sm.tile([128, 8], F32, tag="coll", bufs=2, name=f"coll{hp}")
            den65 = {}
            for h in (ha, hb):
                d65 = sm.tile([65, 512], F32, tag="den", bufs=4, name=f"den{h}")
                nc.vector.tensor_copy(out=d65[64:65, :], in_=av[h][64:65, :])
                nc.sync.dma_start(
                    out=coll[:, 4 * (h % 2) : 4 * (h % 2) + 4],
                    in_=d65[64:65, :],
                )
                den65[h] = d65
            collr = sm.tile([128, 8], F32, tag="collr", bufs=2, name=f"cr{hp}")
            nc.vector.reciprocal(out=collr, in_=coll)
            for h in (ha, hb):
                denr = sm.tile([1, 512], F32, tag="denr", bufs=4, name=f"dr{h}")
                nc.sync.dma_start(
                    out=denr,
                    in_=collr[:, 4 * (h % 2) : 4 * (h % 2) + 4],
                )
                den_bc = sm.tile(
                    [64, 512], F32, tag="den_bc", bufs=2, name=f"dbc{h}"
                )
                nc.gpsimd.partition_broadcast(out_ap=den_bc, in_ap=denr)
                if h % 2 == 0:
                    nc.vector.tensor_tensor(
                        out=attnT[0:64, h // 2, :],
                        in0=av[h][0:64, :],
                        in1=den_bc,
                        op=OP.mult,
                    )
                else:
                    half = sm.tile(
                        [64, 512], BF16, tag="half", bufs=2, name=f"hf{h}"
                    )
                    nc.vector.tensor_tensor(
                        out=half, in0=av[h][0:64, :], in1=den_bc, op=OP.mult
                    )
                    nc.sync.dma_start(out=attnT[64:128, h // 2, :], in_=half)

        # ---- projection + residual ----------------------------------
        outsb = mb.tile([128, 4, NQ], F32, tag="mb", name="outsb")
        pp01 = ps2.tile([128, 2, 512], F32, tag="ps_s", bufs=3, name="pp01")
        pp23 = ps2.tile([128, 2, 512], F32, tag="ps_s", bufs=3, name="pp23")
        for ct in range(4):
            for ot in range(4):
                pp = pp01 if ot < 2 else pp23
                nc.tensor.matmul(
                    pp[:, ot % 2, :],
                    lhsT=projwT[:, ct, 128 * ot : 128 * (ot + 1)],
                    rhs=attnT[:, ct, :],
                    start=(ct == 0),
                    stop=(ct == 3),
                    skip_group_check=True,
                )
        for ot in range(4):
            pp = pp01 if ot < 2 else pp23
            nc.vector.scalar_tensor_tensor(
                out=outsb[:, ot, :],
                in0=pp[:, ot % 2, :],
                scalar=projb[:, ot : ot + 1],
                in1=xres[:, ot, :],
                op0=OP.add,
                op1=OP.add,
            )
            nc.sync.dma_start(
                out=out_d[128 * ot : 128 * (ot + 1), :], in_=outsb[:, ot, :]
            )

    nc.finalize()
    return nc
def _host_prep(x, gn_w, gn_b, qkv_w, qkv_b, proj_w, proj_b, rel_emb):
    """Build the 8 per-core input maps."""
    x = np.asarray(x, dtype=np.float32)
    gn_w = np.asarray(gn_w, dtype=np.float32)
    gn_b = np.asarray(gn_b, dtype=np.float32)
    qkv_w = np.asarray(qkv_w, dtype=np.float32)
    qkv_b = np.asarray(qkv_b, dtype=np.float32)
    proj_w = np.asarray(proj_w, dtype=np.float32)
    proj_b = np.asarray(proj_b, dtype=np.float32)
    rel_emb = np.asarray(rel_emb, dtype=np.float32)

    # relative position bias (matches reference._rel_pos_bias, float32 math)
    dd, hh, ww = np.meshgrid(
        np.arange(D), np.arange(H), np.arange(W), indexing="ij"
    )
    coords = np.stack(
        [dd.ravel(), hh.ravel(), ww.ravel()], axis=-1
    ).astype(np.float32)
    rel = coords[:, None, :] - coords[None, :, :]
    dist = np.sqrt(np.sum(rel * rel, axis=-1, dtype=np.float32)).astype(np.float32)
    buckets = np.clip(
        np.floor(dist / np.float32(MAX_DIST / NUM_BUCKETS)).astype(np.int32),
        0,
        NUM_BUCKETS - 1,
    )
    expb = np.exp(rel_emb[buckets]).astype(np.float32)  # [N, N], symmetric

    import ml_dtypes

    bf16 = ml_dtypes.bfloat16
    projb_eff = (proj_b + proj_w @ qkv_b[2 * C : 3 * C]).astype(np.float32)
    qkvwT = np.ascontiguousarray(qkv_w.T).astype(bf16)
    projwT = np.ascontiguousarray(proj_w.T).astype(bf16)
    gsel = np.zeros((C, GROUPS), np.float32)
    gsel[np.arange(C), np.arange(C) // 64] = 1.0
    gselT = np.ascontiguousarray(gsel.T)
    ones8 = np.ones((128, HEADS), np.float32).astype(bf16)

    xb = x.reshape(B, C, N)
    in_maps = []
    for c in range(NCORES):
        b, qoff = c // 4, (c % 4) * NQ
        xroll = np.roll(xb[b], -qoff, axis=1)
        xc = np.ascontiguousarray(xroll).astype(bf16)
        xres_c = np.ascontiguousarray(xroll[:, 0:NQ])
        bias_c = np.ascontiguousarray(
            np.roll(expb, -qoff, axis=0)[:, qoff : qoff + NQ]
        ).astype(bf16)
        in_maps.append(
            {
                "x": xc,
                "xres": xres_c,
                "qkvwT": qkvwT,
                "projwT": projwT,
                "expbT": bias_c,
                "gnw": gn_w,
                "gnb": gn_b,
                "qkvb": qkv_b,
                "projb": projb_eff,
                "gsel": gsel,
                "gselT": gselT,
                "ones8": ones8,
            }
        )
    return in_maps


def _run(inputs, trace=False, trace_cores=None):
    if "nc" not in _CACHE:
        _CACHE["nc"] = _build()
    nc = _CACHE["nc"]
    in_maps = _host_prep(**inputs)
    last_err = None
    for attempt in range(3):
        try:
            res = run_bass_kernel_spmd(
                nc,
                in_maps,
                core_ids=list(range(NCORES)),
                trace=trace,
                trace_cores=trace_cores,
            )
            break
        except Exception as e:  # transient NRT device errors on first exec
            last_err = e
            import time as _time

            _time.sleep(2.0)
            try:
                import jax

                jax.clear_backends()
            except Exception:
                pass
    else:
        raise last_err
    out = np.empty((B, C, N), np.float32)
    for c in range(NCORES):
        b, qoff = c // 4, (c % 4) * NQ
        out[b][:, qoff : qoff + NQ] = res.results[c]["out"]
    return out.reshape(B, C, D, H, W), res


def kernel(**inputs) -> np.ndarray:
    out, _ = _run(inputs, trace=False)
    return out


# revision 34
# speedup vs baseline: 1.0524x; 1.0524x over previous
"""AttentionBlock3D kernel for 8 Trainium2 NeuronCores (Bass/Tile, SPMD).

Sharding: core c in 0..7 handles batch b = c//4 and query slice
qoff = (c%4)*512 of the N=2048 flattened positions. Each core computes
GroupNorm + full K/V for its batch (replicated across the 4 cores sharing a
batch -> zero cross-core communication), attention for its 512 queries over
all 2048 keys, projection and residual. Host gathers by pure concatenation.

The kernel works in a "transposed" attention layout: scoresT[j, i] (keys on
partitions, queries on the free axis) so that the softmax denominator comes
for free out of the PE via a ones-column appended to V, and no transposes of
the probability matrix are needed (softmax needs no max-subtraction: scores
are O(1) for this block). The [N, N] relative-position bias enters as
exp(bias) (host-gathered from rel_emb, bf16): et = exp(qk/sqrt(d)) * exp(b)
multiplied on the vector engine in 2x bf16 mode. Softmax denominators are
repacked [1,512]->[128,4] by DMA so one cheap reciprocal serves each head
pair, then broadcast across partitions on GpSimd. Matmul operands are bf16
(the GroupNorm statistics path stays float32/float32r); accumulation is fp32.
K projections for later head-pairs and the V projection are interleaved into
the attention stream to keep the PE busy while ACT/DVE chew exp/multiplies.

Per-core inputs are rotated along the position axis by -qoff so that one
SPMD program (query slice = columns 0:512) serves all cores; GroupNorm and
softmax are permutation-invariant so results are unaffected.
"""
import sys

sys.path.insert(0, "/opt/trn_rl_repo")

from contextlib import ExitStack

import numpy as np

import concourse.bacc as bacc
import concourse.mybir as mybir
import concourse.tile as tile
from concourse.bass_utils import run_bass_kernel_spmd

B, C, D, H, W = 2, 512, 8, 16, 16
N = D * H * W  # 2048
HEADS, HD = 8, 64
GROUPS = 8
NUM_BUCKETS = 32
MAX_DIST = 128.0
EPS = 1e-5
NCORES = 8
NQ = N // 4  # 512 queries per core
F32 = mybir.dt.float32
F32R = mybir.dt.float32r
BF16 = mybir.dt.bfloat16

_CACHE = {}


def _build():
    nc = bacc.Bacc(
        "TRN2", target_bir_lowering=False, debug=False, num_devices=NCORES
    )
    AF = mybir.ActivationFunctionType
    OP = mybir.AluOpType

    x_d = nc.dram_tensor("x", [C, N], BF16, kind="ExternalInput").ap()
    xres_d = nc.dram_tensor("xres", [C, NQ], F32, kind="ExternalInput").ap()
    qkvwT_d = nc.dram_tensor("qkvwT", [C, 3 * C], BF16, kind="ExternalInput").ap()
    projwT_d = nc.dram_tensor("projwT", [C, C], BF16, kind="ExternalInput").ap()
    bias_d = nc.dram_tensor("expbT", [N, NQ], BF16, kind="ExternalInput").ap()
    gnw_d = nc.dram_tensor("gnw", [C], F32, kind="ExternalInput").ap()
    gnb_d = nc.dram_tensor("gnb", [C], F32, kind="ExternalInput").ap()
    qkvb_d = nc.dram_tensor("qkvb", [3 * C], F32, kind="ExternalInput").ap()
    projb_d = nc.dram_tensor("projb", [C], F32, kind="ExternalInput").ap()
    gsel_d = nc.dram_tensor("gsel", [C, GROUPS], F32R, kind="ExternalInput").ap()
    gselT_d = nc.dram_tensor("gselT", [GROUPS, C], F32R, kind="ExternalInput").ap()
    ones8_d = nc.dram_tensor("ones8", [128, HEADS], BF16, kind="ExternalInput").ap()
    out_d = nc.dram_tensor("out", [C, NQ], F32, kind="ExternalOutput").ap()

    with tile.TileContext(nc) as tc, ExitStack() as ctx:
        mb = ctx.enter_context(tc.tile_pool(name="mb", bufs=15))
        vg = ctx.enter_context(tc.tile_pool(name="vg", bufs=1))
        ex = ctx.enter_context(tc.tile_pool(name="ex", bufs=1))
        sm = ctx.enter_context(tc.tile_pool(name="sm", bufs=1))
        one = ctx.enter_context(tc.tile_pool(name="one", bufs=1))
        ps2 = ctx.enter_context(tc.tile_pool(name="ps2", bufs=1, space="PSUM"))
        ps1 = ctx.enter_context(tc.tile_pool(name="ps1", bufs=1, space="PSUM"))

        # ---- x load --------------------------------------------------
        xh = []
        for t in range(4):
            xt = mb.tile([128, N], BF16, tag="mb", name=f"xh{t}")
            nc.sync.dma_start(out=xt, in_=x_d[128 * t : 128 * (t + 1), :])
            xh.append(xt)

        # pre-warm ACT table sets off the critical path: Sqrt now (during
        # the x DMA), Exp after GroupNorm (during the qkv phase)
        warm = one.tile([1, 1], F32)
        nc.vector.memset(warm, 1.0)
        warm_eps = one.tile([1, 1], F32)
        nc.vector.memset(warm_eps, 0.0)
        nc.scalar.activation(
            out=warm, in_=warm, func=AF.Sqrt, bias=warm_eps, scale=1.0
        )
        gsel = one.tile([128, 4, GROUPS], F32R)
        nc.sync.dma_start(out=gsel, in_=gsel_d.rearrange("(a p) g -> p a g", p=128))
        gselT = one.tile([GROUPS, C], F32R)
        nc.sync.dma_start(out=gselT, in_=gselT_d)
        ones8 = one.tile([128, HEADS], BF16)
        nc.sync.dma_start(out=ones8, in_=ones8_d)
        gnw = one.tile([128, 4], F32)
        nc.sync.dma_start(out=gnw, in_=gnw_d.rearrange("(a p) -> p a", p=128))
        gnb = one.tile([128, 4], F32)
        nc.sync.dma_start(out=gnb, in_=gnb_d.rearrange("(a p) -> p a", p=128))
        qkvb = one.tile([128, 12], F32)  # col 4*s+t = channels [s*512+128t..+128)
        nc.sync.dma_start(
            out=qkvb, in_=qkvb_d.rearrange("(s a p) -> p (s a)", p=128, s=3)
        )
        projb = one.tile([128, 4], F32)
        nc.sync.dma_start(out=projb, in_=projb_d.rearrange("(a p) -> p a", p=128))

        # ---- constants / weights -------------------------------------
        # qkv weight, split per projection: w[s][:, ct, o] for s in (q, k, v)
        wqkv = []
        for s in range(3):
            ws = mb.tile([128, 4, C], BF16, tag="mb", name=f"w{'qkv'[s]}")
            nc.sync.dma_start(
                out=ws,
                in_=qkvwT_d[:, C * s : C * (s + 1)].rearrange(
                    "(a p) o -> p a o", p=128
                ),
            )
            wqkv.append(ws)
        wq, wk, wv = wqkv
        # ---- GroupNorm ----------------------------------------------
        # Per-channel (mean, E[x^2]) via bn_stats, then group-reduce across
        # partitions with a one-hot matmul, rsqrt, broadcast back, affine.
        ps_g = ps2.tile([128, 512], F32, tag="ps_s", bufs=3, name="ps_g")
        ms_list = []
        for t in range(4):
            stats = sm.tile([128, 4, 6], F32, tag="stats", bufs=4, name=f"st{t}")
            for sg in range(4):
                nc.vector.bn_stats(
                    out=stats[:, sg, :], in_=xh[t][:, 512 * sg : 512 * (sg + 1)]
                )
            mv = sm.tile([128, 2], F32, tag="mv", bufs=2, name=f"mv{t}")
            nc.vector.bn_aggr(out=mv, in_=stats)
            ms = sm.tile([128, 2], F32R, tag="ms", bufs=4, name=f"ms{t}")
            nc.vector.tensor_copy(out=ms[:, 0:1], in_=mv[:, 0:1])
            nc.vector.tensor_tensor(
                out=ms[:, 1:2], in0=mv[:, 0:1], in1=mv[:, 0:1], op=OP.mult
            )
            nc.vector.tensor_tensor(
                out=ms[:, 1:2], in0=ms[:, 1:2], in1=mv[:, 1:2], op=OP.add
            )
            ms_list.append(ms)
            nc.tensor.matmul(
                ps_g[0:GROUPS, 0:2],
                lhsT=gsel[:, t, :],
                rhs=ms,
                start=(t == 0),
                stop=(t == 3),
            )
        gsc = sm.tile([GROUPS, 2], F32, name="gsc")  # (mu_g, E[x^2]_g)
        nc.vector.tensor_scalar_mul(
            out=gsc, in0=ps_g[0:GROUPS, 0:2], scalar1=1.0 / 64.0
        )
        var = sm.tile([GROUPS, 1], F32, name="var")
        nc.vector.tensor_tensor(
            out=var, in0=gsc[:, 0:1], in1=gsc[:, 0:1], op=OP.mult
        )
        nc.vector.tensor_tensor(
            out=var, in0=gsc[:, 1:2], in1=var, op=OP.subtract
        )
        eps_t = sm.tile([GROUPS, 1], F32, name="eps_t")
        nc.vector.memset(eps_t, EPS)
        sd = sm.tile([GROUPS, 1], F32, name="sd")
        nc.scalar.activation(out=sd, in_=var, func=AF.Sqrt, bias=eps_t, scale=1.0)
        rstd = sm.tile([GROUPS, 1], F32, name="rstd")
        nc.vector.reciprocal(out=rstd, in_=sd)
        grhs = sm.tile([GROUPS, 2], F32R, name="grhs")
        nc.vector.tensor_copy(out=grhs[:, 0:1], in_=rstd)
        nc.vector.tensor_copy(out=grhs[:, 1:2], in_=gsc[:, 0:1])

        h_r = []
        for t in range(4):
            ps_b = ps2.tile([128, 512], F32, tag="ps_s", bufs=3, name=f"ps_b{t}")
            nc.tensor.matmul(
                ps_b[:, 0:2],
                lhsT=gselT[:, 128 * t : 128 * (t + 1)],
                rhs=grhs,
                start=True,
                stop=True,
            )
            a_c = sm.tile([128, 1], F32, tag="a_c", bufs=4, name=f"a_c{t}")
            nc.vector.tensor_tensor(
                out=a_c, in0=gnw[:, t : t + 1], in1=ps_b[:, 0:1], op=OP.mult
            )
            b_c = sm.tile([128, 1], F32, tag="b_c", bufs=4, name=f"b_c{t}")
            nc.vector.tensor_tensor(
                out=b_c, in0=ps_b[:, 1:2], in1=a_c, op=OP.mult
            )
            nc.vector.tensor_tensor(
                out=b_c, in0=gnb[:, t : t + 1], in1=b_c, op=OP.subtract
            )
            ht = mb.tile([128, N], BF16, tag="mb", name=f"h{t}")
            nc.vector.tensor_scalar(
                out=ht,
                in0=xh[t],
                scalar1=a_c,
                scalar2=b_c,
                op0=OP.mult,
                op1=OP.add,
            )
            h_r.append(ht)

        warm2 = one.tile([1, 1], F32)
        nc.vector.memset(warm2, 1.0)
        nc.scalar.activation(out=warm2, in_=warm2, func=AF.Exp, scale=1.0)
        # bias tiles (4 x [128, 4, 512]); tile q holds j-blocks 4q..4q+3
        bias_t = []
        for q in range(4):
            bt = mb.tile([128, 4, NQ], BF16, tag="mb", name=f"bias{q}")
            nc.sync.dma_start(
                out=bt,
                in_=bias_d[512 * q : 512 * (q + 1), :].rearrange(
                    "(a p) i -> p a i", p=128
                ),
            )
            bias_t.append(bt)

        # ---- qkv projections ----------------------------------------
        # qT: [q-channel, i] for the 512 local queries (columns 0:512)
        qt = mb.tile([128, 4, NQ], BF16, tag="mb", name="qt")
        for op2 in range(2):
            pq = ps2.tile([128, 2, 512], F32, tag="ps_s", bufs=3, name=f"pq{op2}")
            for oh in range(2):
                for ct in range(4):
                    nc.tensor.matmul(
                        pq[:, oh, :],
                        lhsT=wq[:, ct, 256 * op2 + 128 * oh : 256 * op2 + 128 * oh + 128],
                        rhs=h_r[ct][:, 0:NQ],
                        start=(ct == 0),
                        stop=(ct == 3),
                        skip_group_check=True,
                    )
            # per-partition bias differs across the two o-tiles -> two ops
            for oh in range(2):
                nc.scalar.activation(
                    out=qt[:, 2 * op2 + oh, :],
                    in_=pq[:, oh, :],
                    func=AF.Identity,
                    bias=qkvb[:, 2 * op2 + oh : 2 * op2 + oh + 1],
                    scale=1.0,
                )
        # kT: [k-channel, j] over all 2048 keys. Only the pair-0 tile is
        # emitted up front; later pairs' K projections are interleaved into
        # the attention stream (they fill PE bubbles / pair-boundary tails).
        kt = {}

        def emit_kt(ot):
            ktile = mb.tile([128, N], BF16, tag="mb", name=f"kt{ot}")
            for njp in range(2):
                pk = ps2.tile(
                    [128, 2, 512], F32, tag="ps_s", bufs=3, name=f"pk{ot}{njp}"
                )
                for nh in range(2):
                    for ct in range(4):
                        nc.tensor.matmul(
                            pk[:, nh, :],
                            lhsT=wk[:, ct, 128 * ot : 128 * (ot + 1)],
                            rhs=h_r[ct][
                                :, 1024 * njp + 512 * nh : 1024 * njp + 512 * nh + 512
                            ],
                            start=(ct == 0),
                            stop=(ct == 3),
                            skip_group_check=True,
                        )
                nc.scalar.activation(
                    out=ktile[:, 1024 * njp : 1024 * (njp + 1)],
                    in_=pk.rearrange("p a i -> p (a i)"),
                    func=AF.Identity,
                    bias=qkvb[:, 4 + ot : 4 + ot + 1],
                    scale=1.0,
                )
            kt[ot] = ktile

        for _ot in range(2):
            emit_kt(_ot)
        # V in [j, v-channel] layout, augmented with a ones column per head:
        # vaug[:, nt, h*65+d] = v[j, 64h+d] ; vaug[:, nt, h*65+64] = 1.
        # The v-bias is folded into the projection bias on the host
        # (softmax weights sum to 1, so attn(v+vb) = attn(v) + vb).
        vaug = []
        for q in range(4):
            vt = vg.tile([128, 4, HEADS, 65], BF16, name=f"vaug{q}")
            nc.vector.tensor_copy(
                out=vt[:, :, :, 64:65].squeeze(3),
                in_=ones8.unsqueeze(1).broadcast_to([128, 4, HEADS]),
            )
            vaug.append(vt)
        def emit_v_chunk(ntp):
            pv = ps2.tile([128, 2, 512], F32, tag="ps_s", bufs=3, name=f"pv{ntp}")
            for nh in range(2):
                nt = 2 * ntp + nh
                for ct in range(4):
                    nc.tensor.matmul(
                        pv[:, nh, :],
                        lhsT=h_r[ct][:, 128 * nt : 128 * (nt + 1)],
                        rhs=wv[:, ct, :],
                        start=(ct == 0),
                        stop=(ct == 3),
                        skip_group_check=True,
                    )
            q, jj = (2 * ntp) // 4, (2 * ntp) % 4
            nc.vector.tensor_copy(
                out=vaug[q][:, jj : jj + 2, :, 0:64],
                in_=pv.rearrange("p a (h d) -> p a h d", d=HD),
            )

        # ---- attention (head pairs; QK row-packed at base 0/64) -----
        # preload projection weights + residual so proj can start immediately
        projwT = mb.tile([128, 4, C], BF16, tag="mb", name="projwT")
        nc.sync.dma_start(
            out=projwT, in_=projwT_d.rearrange("(a p) o -> p a o", p=128)
        )
        xres = mb.tile([128, 4, NQ], F32, tag="mb", name="xres")
        nc.sync.dma_start(
            out=xres, in_=xres_d.rearrange("(a p) i -> p a i", p=128)
        )
        attnT = mb.tile([128, 4, NQ], BF16, tag="mb", name="attnT")
        for hp in range(4):
            ha, hb = 2 * hp, 2 * hp + 1
            av = {}
            for h, lab in ((ha, "a"), (hb, "b")):
                av[h] = ps1.tile(
                    [128, 512], F32, tag="ps_av", bufs=2, name=f"av{h}"
                )
            pend = []  # delayed AV emission: (h, g, et)
            for g in range(8):
                if hp == 0:
                    emit_v_chunk(g)
                TA = ps2.tile(
                    [128, 2, 512], F32, tag="ps_s", bufs=3, name=f"sa{hp}_{g}"
                )
                TB = ps2.tile(
                    [128, 2, 512], F32, tag="ps_s", bufs=3, name=f"sb{hp}_{g}"
                )
                for jj in range(2):
                    jb = 2 * g + jj
                    js = slice(128 * jb, 128 * (jb + 1))
                    # the two K=64 matmuls run concurrently (row groups 0/64)
                    nc.tensor.matmul(
                        TA[:, jj, :],
                        lhsT=kt[hp][0:64, js],
                        rhs=qt[0:64, hp, :],
                        start=True,
                        stop=True,
                        skip_group_check=True,
                    )
                    nc.tensor.matmul(
                        TB[:, jj, :],
                        lhsT=kt[hp][64:128, js],
                        rhs=qt[64:128, hp, :],
                        start=True,
                        stop=True,
                        skip_group_check=True,
                    )
                new_pend = []
                for h, T in ((ha, TA), (hb, TB)):
                    etr = ex.tile(
                        [128, 2, 512], BF16, tag="etr", bufs=6, name=f"er{h}_{g}"
                    )
                    nc.scalar.activation(out=etr, in_=T, func=AF.Exp, scale=0.125)
                    et = ex.tile(
                        [128, 2, 512], BF16, tag="et", bufs=10, name=f"et{h}_{g}"
                    )
                    jb0 = 2 * g
                    nc.vector.tensor_tensor(
                        out=et,
                        in0=etr,
                        in1=bias_t[jb0 // 4][:, jb0 % 4 : jb0 % 4 + 2, :],
                        op=OP.mult,
                    )
                    new_pend.append((h, g, et))
                pend = pend + new_pend
                depth = 6  # entries = 2 per delayed group
                while len(pend) > depth:
                    h, gp, etp = pend.pop(0)
                    for jj in range(2):
                        jb = 2 * gp + jj
                        nc.tensor.matmul(
                            av[h][0:65, :],
                            lhsT=vaug[jb // 4][:, jb % 4, h, :],
                            rhs=etp[:, jj, :],
                            start=(gp == 0 and jj == 0),
                            stop=(gp == 7 and jj == 1),
                            skip_group_check=True,
                        )
            for h, gp, etp in pend:
                for jj in range(2):
                    jb = 2 * gp + jj
                    nc.tensor.matmul(
                        av[h][0:65, :],
                        lhsT=vaug[jb // 4][:, jb % 4, h, :],
                        rhs=etp[:, jj, :],
                        start=(gp == 0 and jj == 0),
                        stop=(gp == 7 and jj == 1),
                        skip_group_check=True,
                    )
            if hp + 2 < 4:
                emit_kt(hp + 2)
            # normalize: rows 0:63 = unnormalized attn^T, row 64 = denom.
            # Reciprocal free-dim cost is brutal (~6 ns/elem), so repack the
            # two denominator rows into [128, 8] via DMA, reciprocal once,
            # and DMA back for the PE broadcast matmul.
            coll = sm.tile([128, 8], F32, tag="coll", bufs=2, name=f"coll{hp}")
            den65 = {}
            for h in (ha, hb):
                d65 = sm.tile([65, 512], F32, tag="den", bufs=4, name=f"den{h}")
                nc.vector.tensor_copy(out=d65[64:65, :], in_=av[h][64:65, :])
                nc.sync.dma_start(
                    out=coll[:, 4 * (h % 2) : 4 * (h % 2) + 4],
                    in_=d65[64:65, :],
                )
                den65[h] = d65
            collr = sm.tile([128, 8], F32, tag="collr", bufs=2, name=f"cr{hp}")
            nc.vector.reciprocal(out=collr, in_=coll)
            for h in (ha, hb):
                denr = sm.tile([1, 512], F32, tag="denr", bufs=4, name=f"dr{h}")
                nc.sync.dma_start(
                    out=denr,
                    in_=collr[:, 4 * (h % 2) : 4 * (h % 2) + 4],
                )
                den_bc = sm.tile(
                    [64, 512], F32, tag="den_bc", bufs=2, name=f"dbc{h}"
                )
                nc.gpsimd.partition_broadcast(out_ap=den_bc, in_ap=denr)
                if h % 2 == 0:
                    nc.vector.tensor_tensor(
                        out=attnT[0:64, h // 2, :],
                        in0=av[h][0:64, :],
                        in1=den_bc,
                        op=OP.mult,
                    )
                else:
                    half = sm.tile(
                        [64, 512], BF16, tag="half", bufs=2, name=f"hf{h}"
                    )
                    nc.vector.tensor_tensor(
                        out=half, in0=av[h][0:64, :], in1=den_bc, op=OP.mult
                    )
                    nc.sync.dma_start(out=attnT[64:128, h // 2, :], in_=half)

        # ---- projection + residual ----------------------------------
        outsb = mb.tile([128, 4, NQ], F32, tag="mb", name="outsb")
        pp01 = ps2.tile([128, 2, 512], F32, tag="ps_s", bufs=3, name="pp01")
        pp23 = ps2.tile([128, 2, 512], F32, tag="ps_s", bufs=3, name="pp23")
        for ct in range(4):
            for ot in range(4):
                pp = pp01 if ot < 2 else pp23
                nc.tensor.matmul(
                    pp[:, ot % 2, :],
                    lhsT=projwT[:, ct, 128 * ot : 128 * (ot + 1)],
                    rhs=attnT[:, ct, :],
                    start=(ct == 0),
                    stop=(ct == 3),
                    skip_group_check=True,
                )
        for ot in range(4):
            pp = pp01 if ot < 2 else pp23
            nc.vector.scalar_tensor_tensor(
                out=outsb[:, ot, :],
                in0=pp[:, ot % 2, :],
                scalar=projb[:, ot : ot + 1],
                in1=xres[:, ot, :],
                op0=OP.add,
                op1=OP.add,
            )
            nc.sync.dma_start(
                out=out_d[128 * ot : 128 * (ot + 1), :], in_=outsb[:, ot, :]
            )

    nc.finalize()
    return nc
def _host_prep(x, gn_w, gn_b, qkv_w, qkv_b, proj_w, proj_b, rel_emb):
    """Build the 8 per-core input maps."""
    x = np.asarray(x, dtype=np.float32)
    gn_w = np.asarray(gn_w, dtype=np.float32)
    gn_b = np.asarray(gn_b, dtype=np.float32)
    qkv_w = np.asarray(qkv_w, dtype=np.float32)
    qkv_b = np.asarray(qkv_b, dtype=np.float32)
    proj_w = np.asarray(proj_w, dtype=np.float32)
    proj_b = np.asarray(proj_b, dtype=np.float32)
    rel_emb = np.asarray(rel_emb, dtype=np.float32)

    # relative position bias (matches reference._rel_pos_bias, float32 math)
    dd, hh, ww = np.meshgrid(
        np.arange(D), np.arange(H), np.arange(W), indexing="ij"
    )
    coords = np.stack(
        [dd.ravel(), hh.ravel(), ww.ravel()], axis=-1
    ).astype(np.float32)
    rel = coords[:, None, :] - coords[None, :, :]
    dist = np.sqrt(np.sum(rel * rel, axis=-1, dtype=np.float32)).astype(np.float32)
    buckets = np.clip(
        np.floor(dist / np.float32(MAX_DIST / NUM_BUCKETS)).astype(np.int32),
        0,
        NUM_BUCKETS - 1,
    )
    expb = np.exp(rel_emb[buckets]).astype(np.float32)  # [N, N], symmetric

    import ml_dtypes

    bf16 = ml_dtypes.bfloat16
    projb_eff = (proj_b + proj_w @ qkv_b[2 * C : 3 * C]).astype(np.float32)
    qkvwT = np.ascontiguousarray(qkv_w.T).astype(bf16)
    projwT = np.ascontiguousarray(proj_w.T).astype(bf16)
    gsel = np.zeros((C, GROUPS), np.float32)
    gsel[np.arange(C), np.arange(C) // 64] = 1.0
    gselT = np.ascontiguousarray(gsel.T)
    ones8 = np.ones((128, HEADS), np.float32).astype(bf16)

    xb = x.reshape(B, C, N)
    in_maps = []
    for c in range(NCORES):
        b, qoff = c // 4, (c % 4) * NQ
        xroll = np.roll(xb[b], -qoff, axis=1)
        xc = np.ascontiguousarray(xroll).astype(bf16)
        xres_c = np.ascontiguousarray(xroll[:, 0:NQ])
        bias_c = np.ascontiguousarray(
            np.roll(expb, -qoff, axis=0)[:, qoff : qoff + NQ]
        ).astype(bf16)
        in_maps.append(
            {
                "x": xc,
                "xres": xres_c,
                "qkvwT": qkvwT,
                "projwT": projwT,
                "expbT": bias_c,
                "gnw": gn_w,
                "gnb": gn_b,
                "qkvb": qkv_b,
                "projb": projb_eff,
                "gsel": gsel,
                "gselT": gselT,
                "ones8": ones8,
            }
        )
    return in_maps


def _run(inputs, trace=False, trace_cores=None):
    if "nc" not in _CACHE:
        _CACHE["nc"] = _build()
    nc = _CACHE["nc"]
    in_maps = _host_prep(**inputs)
    last_err = None
    for attempt in range(3):
        try:
            res = run_bass_kernel_spmd(
                nc,
                in_maps,
                core_ids=list(range(NCORES)),
                trace=trace,
                trace_cores=trace_cores,
            )
            break
        except Exception as e:  # transient NRT device errors on first exec
            last_err = e
            import time as _time

            _time.sleep(2.0)
            try:
                import jax

                jax.clear_backends()
            except Exception:
                pass
    else:
        raise last_err
    out = np.empty((B, C, N), np.float32)
    for c in range(NCORES):
        b, qoff = c // 4, (c % 4) * NQ
        out[b][:, qoff : qoff + NQ] = res.results[c]["out"]
    return out.reshape(B, C, D, H, W), res


def kernel(**inputs) -> np.ndarray:
    out, _ = _run(inputs, trace=False)
    return out


# revision 36
# speedup vs baseline: 1.2607x; 1.1979x over previous
"""AttentionBlock3D kernel for 8 Trainium2 NeuronCores (Bass/Tile, SPMD).

Sharding: core c in 0..7 handles batch b = c//4 and query slice
qoff = (c%4)*512 of the N=2048 flattened positions. Each core computes
GroupNorm + full K/V for its batch (replicated across the 4 cores sharing a
batch -> zero cross-core communication), attention for its 512 queries over
all 2048 keys, projection and residual. Host gathers by pure concatenation.

The kernel works in a "transposed" attention layout: scoresT[j, i] (keys on
partitions, queries on the free axis) so that the softmax denominator comes
for free out of the PE via a ones-column appended to V, and no transposes of
the probability matrix are needed (softmax needs no max-subtraction: scores
are O(1) for this block). The [N, N] relative-position bias enters as
exp(bias) (host-gathered from rel_emb, bf16): et = exp(qk/sqrt(d)) * exp(b)
multiplied on the vector engine in 2x bf16 mode. Softmax denominators are
repacked [1,512]->[128,4] by DMA so one cheap reciprocal serves each head
pair, then broadcast across partitions on GpSimd. Matmul operands are bf16
(the GroupNorm statistics path stays float32/float32r); accumulation is fp32.
K projections for later head-pairs and the V projection are interleaved into
the attention stream to keep the PE busy while ACT/DVE chew exp/multiplies.

Per-core inputs are rotated along the position axis by -qoff so that one
SPMD program (query slice = columns 0:512) serves all cores; GroupNorm and
softmax are permutation-invariant so results are unaffected.
"""
import sys

sys.path.insert(0, "/opt/trn_rl_repo")

from contextlib import ExitStack

import numpy as np

import concourse.bacc as bacc
import concourse.mybir as mybir
import concourse.tile as tile
from concourse.bass_utils import run_bass_kernel_spmd

B, C, D, H, W = 2, 512, 8, 16, 16
N = D * H * W  # 2048
HEADS, HD = 8, 64
GROUPS = 8
NUM_BUCKETS = 32
MAX_DIST = 128.0
EPS = 1e-5
NCORES = 8
NQ = N // 4  # 512 queries per core
F32 = mybir.dt.float32
F32R = mybir.dt.float32r
BF16 = mybir.dt.bfloat16

_CACHE = {}


def _build():
    nc = bacc.Bacc(
        "TRN2", target_bir_lowering=False, debug=False, num_devices=NCORES
    )
    AF = mybir.ActivationFunctionType
    OP = mybir.AluOpType

    x_d = nc.dram_tensor("x", [C, N], BF16, kind="ExternalInput").ap()
    xres_d = nc.dram_tensor("xres", [C, NQ], F32, kind="ExternalInput").ap()
    qkvwT_d = nc.dram_tensor("qkvwT", [C, 3 * C], BF16, kind="ExternalInput").ap()
    projwT_d = nc.dram_tensor("projwT", [C, C], BF16, kind="ExternalInput").ap()
    bias_d = nc.dram_tensor("expbT", [N, NQ], BF16, kind="ExternalInput").ap()
    gnw_d = nc.dram_tensor("gnw", [C], F32, kind="ExternalInput").ap()
    gnb_d = nc.dram_tensor("gnb", [C], F32, kind="ExternalInput").ap()
    qkvb_d = nc.dram_tensor("qkvb", [3 * C], F32, kind="ExternalInput").ap()
    projb_d = nc.dram_tensor("projb", [C], F32, kind="ExternalInput").ap()
    gsel_d = nc.dram_tensor("gsel", [C, GROUPS], F32R, kind="ExternalInput").ap()
    gselT_d = nc.dram_tensor("gselT", [GROUPS, C], F32R, kind="ExternalInput").ap()
    ones8_d = nc.dram_tensor("ones8", [128, HEADS], BF16, kind="ExternalInput").ap()
    out_d = nc.dram_tensor("out", [C, NQ], F32, kind="ExternalOutput").ap()

    with tile.TileContext(nc) as tc, ExitStack() as ctx:
        mb = ctx.enter_context(tc.tile_pool(name="mb", bufs=15))
        vg = ctx.enter_context(tc.tile_pool(name="vg", bufs=1))
        ex = ctx.enter_context(tc.tile_pool(name="ex", bufs=1))
        sm = ctx.enter_context(tc.tile_pool(name="sm", bufs=1))
        one = ctx.enter_context(tc.tile_pool(name="one", bufs=1))
        ps2 = ctx.enter_context(tc.tile_pool(name="ps2", bufs=1, space="PSUM"))
        ps1 = ctx.enter_context(tc.tile_pool(name="ps1", bufs=1, space="PSUM"))

        # ---- x load --------------------------------------------------
        xh = []
        for t in range(4):
            xt = mb.tile([128, N], BF16, tag="mb", name=f"xh{t}")
            nc.sync.dma_start(out=xt, in_=x_d[128 * t : 128 * (t + 1), :])
            xh.append(xt)

        # pre-warm ACT table sets off the critical path: Sqrt now (during
        # the x DMA), Exp after GroupNorm (during the qkv phase)
        warm = one.tile([1, 1], F32)
        nc.vector.memset(warm, 1.0)
        warm_eps = one.tile([1, 1], F32)
        nc.vector.memset(warm_eps, 0.0)
        nc.scalar.activation(
            out=warm, in_=warm, func=AF.Sqrt, bias=warm_eps, scale=1.0
        )
        gsel = one.tile([128, 4, GROUPS], F32R)
        nc.sync.dma_start(out=gsel, in_=gsel_d.rearrange("(a p) g -> p a g", p=128))
        gselT = one.tile([GROUPS, C], F32R)
        nc.sync.dma_start(out=gselT, in_=gselT_d)
        ones8 = one.tile([128, HEADS], BF16)
        nc.sync.dma_start(out=ones8, in_=ones8_d)
        gnw = one.tile([128, 4], F32)
        nc.sync.dma_start(out=gnw, in_=gnw_d.rearrange("(a p) -> p a", p=128))
        gnb = one.tile([128, 4], F32)
        nc.sync.dma_start(out=gnb, in_=gnb_d.rearrange("(a p) -> p a", p=128))
        qkvb = one.tile([128, 12], F32)  # col 4*s+t = channels [s*512+128t..+128)
        nc.sync.dma_start(
            out=qkvb, in_=qkvb_d.rearrange("(s a p) -> p (s a)", p=128, s=3)
        )
        projb = one.tile([128, 4], F32)
        nc.sync.dma_start(out=projb, in_=projb_d.rearrange("(a p) -> p a", p=128))

        # ---- constants / weights -------------------------------------
        # qkv weight, split per projection: w[s][:, ct, o] for s in (q, k, v)
        wqkv = []
        for s in range(3):
            ws = mb.tile([128, 4, C], BF16, tag="mb", name=f"w{'qkv'[s]}")
            nc.sync.dma_start(
                out=ws,
                in_=qkvwT_d[:, C * s : C * (s + 1)].rearrange(
                    "(a p) o -> p a o", p=128
                ),
            )
            wqkv.append(ws)
        wq, wk, wv = wqkv
        # ---- GroupNorm ----------------------------------------------
        # Per-channel (mean, E[x^2]) via bn_stats, then group-reduce across
        # partitions with a one-hot matmul, rsqrt, broadcast back, affine.
        ps_g = ps2.tile([128, 512], F32, tag="ps_s", bufs=3, name="ps_g")
        ms_list = []
        for t in range(4):
            stats = sm.tile([128, 4, 6], F32, tag="stats", bufs=4, name=f"st{t}")
            for sg in range(4):
                nc.vector.bn_stats(
                    out=stats[:, sg, :], in_=xh[t][:, 512 * sg : 512 * (sg + 1)]
                )
            mv = sm.tile([128, 2], F32, tag="mv", bufs=2, name=f"mv{t}")
            nc.vector.bn_aggr(out=mv, in_=stats)
            ms = sm.tile([128, 2], F32R, tag="ms", bufs=4, name=f"ms{t}")
            nc.vector.tensor_copy(out=ms[:, 0:1], in_=mv[:, 0:1])
            nc.vector.tensor_tensor(
                out=ms[:, 1:2], in0=mv[:, 0:1], in1=mv[:, 0:1], op=OP.mult
            )
            nc.vector.tensor_tensor(
                out=ms[:, 1:2], in0=ms[:, 1:2], in1=mv[:, 1:2], op=OP.add
            )
            ms_list.append(ms)
            nc.tensor.matmul(
                ps_g[0:GROUPS, 0:2],
                lhsT=gsel[:, t, :],
                rhs=ms,
                start=(t == 0),
                stop=(t == 3),
            )
        gsc = sm.tile([GROUPS, 2], F32, name="gsc")  # (mu_g, E[x^2]_g)
        nc.vector.tensor_scalar_mul(
            out=gsc, in0=ps_g[0:GROUPS, 0:2], scalar1=1.0 / 64.0
        )
        var = sm.tile([GROUPS, 1], F32, name="var")
        nc.vector.tensor_tensor(
            out=var, in0=gsc[:, 0:1], in1=gsc[:, 0:1], op=OP.mult
        )
        nc.vector.tensor_tensor(
            out=var, in0=gsc[:, 1:2], in1=var, op=OP.subtract
        )
        eps_t = sm.tile([GROUPS, 1], F32, name="eps_t")
        nc.vector.memset(eps_t, EPS)
        sd = sm.tile([GROUPS, 1], F32, name="sd")
        nc.scalar.activation(out=sd, in_=var, func=AF.Sqrt, bias=eps_t, scale=1.0)
        rstd = sm.tile([GROUPS, 1], F32, name="rstd")
        nc.vector.reciprocal(out=rstd, in_=sd)
        grhs = sm.tile([GROUPS, 2], F32R, name="grhs")
        nc.vector.tensor_copy(out=grhs[:, 0:1], in_=rstd)
        nc.vector.tensor_copy(out=grhs[:, 1:2], in_=gsc[:, 0:1])

        h_r = []
        for t in range(4):
            ps_b = ps2.tile([128, 512], F32, tag="ps_s", bufs=3, name=f"ps_b{t}")
            nc.tensor.matmul(
                ps_b[:, 0:2],
                lhsT=gselT[:, 128 * t : 128 * (t + 1)],
                rhs=grhs,
                start=True,
                stop=True,
            )
            a_c = sm.tile([128, 1], F32, tag="a_c", bufs=4, name=f"a_c{t}")
            nc.vector.tensor_tensor(
                out=a_c, in0=gnw[:, t : t + 1], in1=ps_b[:, 0:1], op=OP.mult
            )
            b_c = sm.tile([128, 1], F32, tag="b_c", bufs=4, name=f"b_c{t}")
            nc.vector.tensor_tensor(
                out=b_c, in0=ps_b[:, 1:2], in1=a_c, op=OP.mult
            )
            nc.vector.tensor_tensor(
                out=b_c, in0=gnb[:, t : t + 1], in1=b_c, op=OP.subtract
            )
            ht = mb.tile([128, N], BF16, tag="mb", name=f"h{t}")
            nc.vector.tensor_scalar(
                out=ht,
                in0=xh[t],
                scalar1=a_c,
                scalar2=b_c,
                op0=OP.mult,
                op1=OP.add,
            )
            h_r.append(ht)

        warm2 = one.tile([1, 1], F32)
        nc.vector.memset(warm2, 1.0)
        nc.scalar.activation(out=warm2, in_=warm2, func=AF.Exp, scale=1.0)
        # bias tiles (4 x [128, 4, 512]); tile q holds j-blocks 4q..4q+3
        bias_t = []
        for q in range(4):
            bt = mb.tile([128, 4, NQ], BF16, tag="mb", name=f"bias{q}")
            nc.sync.dma_start(
                out=bt,
                in_=bias_d[512 * q : 512 * (q + 1), :].rearrange(
                    "(a p) i -> p a i", p=128
                ),
            )
            bias_t.append(bt)

        # ---- qkv projections ----------------------------------------
        # qT: [q-channel, i] for the 512 local queries (columns 0:512)
        qt = mb.tile([128, 4, NQ], BF16, tag="mb", name="qt")
        for op2 in range(2):
            pq = ps2.tile([128, 2, 512], F32, tag="ps_s", bufs=3, name=f"pq{op2}")
            for oh in range(2):
                for ct in range(4):
                    nc.tensor.matmul(
                        pq[:, oh, :],
                        lhsT=wq[:, ct, 256 * op2 + 128 * oh : 256 * op2 + 128 * oh + 128],
                        rhs=h_r[ct][:, 0:NQ],
                        start=(ct == 0),
                        stop=(ct == 3),
                        skip_group_check=True,
                    )
            # per-partition bias differs across the two o-tiles -> two ops
            for oh in range(2):
                nc.scalar.activation(
                    out=qt[:, 2 * op2 + oh, :],
                    in_=pq[:, oh, :],
                    func=AF.Identity,
                    bias=qkvb[:, 2 * op2 + oh : 2 * op2 + oh + 1],
                    scale=1.0,
                )
        # kT: [k-channel, j] over all 2048 keys. Only the pair-0 tile is
        # emitted up front; later pairs' K projections are interleaved into
        # the attention stream (they fill PE bubbles / pair-boundary tails).
        kt = {}

        def emit_kt(ot):
            ktile = mb.tile([128, N], BF16, tag="mb", name=f"kt{ot}")
            for njp in range(2):
                pk = ps2.tile(
                    [128, 2, 512], F32, tag="ps_s", bufs=3, name=f"pk{ot}{njp}"
                )
                for nh in range(2):
                    for ct in range(4):
                        nc.tensor.matmul(
                            pk[:, nh, :],
                            lhsT=wk[:, ct, 128 * ot : 128 * (ot + 1)],
                            rhs=h_r[ct][
                                :, 1024 * njp + 512 * nh : 1024 * njp + 512 * nh + 512
                            ],
                            start=(ct == 0),
                            stop=(ct == 3),
                            skip_group_check=True,
                        )
                nc.scalar.activation(
                    out=ktile[:, 1024 * njp : 1024 * (njp + 1)],
                    in_=pk.rearrange("p a i -> p (a i)"),
                    func=AF.Identity,
                    bias=qkvb[:, 4 + ot : 4 + ot + 1],
                    scale=1.0,
                )
            kt[ot] = ktile

        for _ot in range(2):
            emit_kt(_ot)
        # V in [j, v-channel] layout, augmented with a ones column per head:
        # vaug[:, nt, h*65+d] = v[j, 64h+d] ; vaug[:, nt, h*65+64] = 1.
        # The v-bias is folded into the projection bias on the host
        # (softmax weights sum to 1, so attn(v+vb) = attn(v) + vb).
        vaug = []
        for q in range(4):
            vt = vg.tile([128, 4, HEADS, 65], BF16, name=f"vaug{q}")
            nc.vector.tensor_copy(
                out=vt[:, :, :, 64:65].squeeze(3),
                in_=ones8.unsqueeze(1).broadcast_to([128, 4, HEADS]),
            )
            vaug.append(vt)
        def emit_v_chunk(ntp):
            pv = ps2.tile([128, 2, 512], F32, tag="ps_s", bufs=3, name=f"pv{ntp}")
            for nh in range(2):
                nt = 2 * ntp + nh
                for ct in range(4):
                    nc.tensor.matmul(
                        pv[:, nh, :],
                        lhsT=h_r[ct][:, 128 * nt : 128 * (nt + 1)],
                        rhs=wv[:, ct, :],
                        start=(ct == 0),
                        stop=(ct == 3),
                        skip_group_check=True,
                    )
            q, jj = (2 * ntp) // 4, (2 * ntp) % 4
            nc.vector.tensor_copy(
                out=vaug[q][:, jj : jj + 2, :, 0:64],
                in_=pv.rearrange("p a (h d) -> p a h d", d=HD),
            )

        # ---- attention (head pairs; QK row-packed at base 0/64) -----
        # preload projection weights + residual so proj can start immediately
        projwT = mb.tile([128, 4, C], BF16, tag="mb", name="projwT")
        nc.sync.dma_start(
            out=projwT, in_=projwT_d.rearrange("(a p) o -> p a o", p=128)
        )
        xres = mb.tile([128, 4, NQ], F32, tag="mb", name="xres")
        nc.sync.dma_start(
            out=xres, in_=xres_d.rearrange("(a p) i -> p a i", p=128)
        )
        attnT = mb.tile([128, 4, NQ], BF16, tag="mb", name="attnT")
        for hp in range(4):
            ha, hb = 2 * hp, 2 * hp + 1
            av = {}
            for h, lab in ((ha, "a"), (hb, "b")):
                av[h] = ps1.tile(
                    [128, 512], F32, tag="ps_av", bufs=2, name=f"av{h}"
                )
            pend = []  # delayed AV emission: (h, g, et)
            for g in range(8):
                if hp == 0:
                    emit_v_chunk(g)
                TA = ps2.tile(
                    [128, 2, 512], F32, tag="ps_s", bufs=3, name=f"sa{hp}_{g}"
                )
                TB = ps2.tile(
                    [128, 2, 512], F32, tag="ps_s", bufs=3, name=f"sb{hp}_{g}"
                )
                for jj in range(2):
                    jb = 2 * g + jj
                    js = slice(128 * jb, 128 * (jb + 1))
                    # the two K=64 matmuls run concurrently (row groups 0/64)
                    nc.tensor.matmul(
                        TA[:, jj, :],
                        lhsT=kt[hp][0:64, js],
                        rhs=qt[0:64, hp, :],
                        start=True,
                        stop=True,
                        skip_group_check=True,
                    )
                    nc.tensor.matmul(
                        TB[:, jj, :],
                        lhsT=kt[hp][64:128, js],
                        rhs=qt[64:128, hp, :],
                        start=True,
                        stop=True,
                        skip_group_check=True,
                    )
                new_pend = []
                for h, T in ((ha, TA), (hb, TB)):
                    etr = ex.tile(
                        [128, 2, 512], BF16, tag="etr", bufs=6, name=f"er{h}_{g}"
                    )
                    nc.scalar.activation(out=etr, in_=T, func=AF.Exp, scale=0.125)
                    et = ex.tile(
                        [128, 2, 512], BF16, tag="et", bufs=10, name=f"et{h}_{g}"
                    )
                    jb0 = 2 * g
                    nc.vector.tensor_tensor(
                        out=et,
                        in0=etr,
                        in1=bias_t[jb0 // 4][:, jb0 % 4 : jb0 % 4 + 2, :],
                        op=OP.mult,
                    )
                    new_pend.append((h, g, et))
                pend = pend + new_pend
                depth = 6  # entries = 2 per delayed group
                while len(pend) > depth:
                    h, gp, etp = pend.pop(0)
                    for jj in range(2):
                        jb = 2 * gp + jj
                        nc.tensor.matmul(
                            av[h][0:65, :],
                            lhsT=vaug[jb // 4][:, jb % 4, h, :],
                            rhs=etp[:, jj, :],
                            start=(gp == 0 and jj == 0),
                            stop=(gp == 7 and jj == 1),
                            skip_group_check=True,
                        )
            for h, gp, etp in pend:
                for jj in range(2):
                    jb = 2 * gp + jj
                    nc.tensor.matmul(
                        av[h][0:65, :],
                        lhsT=vaug[jb // 4][:, jb % 4, h, :],
                        rhs=etp[:, jj, :],
                        start=(gp == 0 and jj == 0),
                        stop=(gp == 7 and jj == 1),
                        skip_group_check=True,
                    )
            if hp + 2 < 4:
                emit_kt(hp + 2)
            # normalize: rows 0:63 = unnormalized attn^T, row 64 = denom.
            # Reciprocal free-dim cost is brutal (~6 ns/elem), so repack the
            # two denominator rows into [128, 8] via DMA, reciprocal once,
            # and DMA back for the PE broadcast matmul.
            coll = sm.tile([128, 8], F32, tag="coll", bufs=2, name=f"coll{hp}")
            den65 = {}
            for h in (ha, hb):
                d65 = sm.tile([65, 512], F32, tag="den", bufs=4, name=f"den{h}")
                nc.vector.tensor_copy(out=d65[64:65, :], in_=av[h][64:65, :])
                nc.sync.dma_start(
                    out=coll[:, 4 * (h % 2) : 4 * (h % 2) + 4],
                    in_=d65[64:65, :],
                )
                den65[h] = d65
            collr = sm.tile([128, 8], F32, tag="collr", bufs=2, name=f"cr{hp}")
            nc.vector.reciprocal(out=collr, in_=coll)
            for h in (ha, hb):
                denr = sm.tile([1, 512], F32, tag="denr", bufs=4, name=f"dr{h}")
                nc.sync.dma_start(
                    out=denr,
                    in_=collr[:, 4 * (h % 2) : 4 * (h % 2) + 4],
                )
                den_bc = sm.tile(
                    [64, 512], F32, tag="den_bc", bufs=2, name=f"dbc{h}"
                )
                nc.gpsimd.partition_broadcast(out_ap=den_bc, in_ap=denr)
                if h % 2 == 0:
                    nc.vector.tensor_tensor(
                        out=attnT[0:64, h // 2, :],
                        in0=av[h][0:64, :],
                        in1=den_bc,
                        op=OP.mult,
                    )
                else:
                    half = sm.tile(
                        [64, 512], BF16, tag="half", bufs=2, name=f"hf{h}"
                    )
                    nc.vector.tensor_tensor(
                        out=half, in0=av[h][0:64, :], in1=den_bc, op=OP.mult
                    )
                    nc.sync.dma_start(out=attnT[64:128, h // 2, :], in_=half)

        # ---- projection + residual ----------------------------------
        outsb = mb.tile([128, 4, NQ], F32, tag="mb", name="outsb")
        pp01 = ps2.tile([128, 2, 512], F32, tag="ps_s", bufs=3, name="pp01")
        pp23 = ps2.tile([128, 2, 512], F32, tag="ps_s", bufs=3, name="pp23")
        for ct in range(4):
            for ot in range(4):
                pp = pp01 if ot < 2 else pp23
                nc.tensor.matmul(
                    pp[:, ot % 2, :],
                    lhsT=projwT[:, ct, 128 * ot : 128 * (ot + 1)],
                    rhs=attnT[:, ct, :],
                    start=(ct == 0),
                    stop=(ct == 3),
                    skip_group_check=True,
                )
        for ot in range(4):
            pp = pp01 if ot < 2 else pp23
            nc.vector.scalar_tensor_tensor(
                out=outsb[:, ot, :],
                in0=pp[:, ot % 2, :],
                scalar=projb[:, ot : ot + 1],
                in1=xres[:, ot, :],
                op0=OP.add,
                op1=OP.add,
            )
            nc.sync.dma_start(
                out=out_d[128 * ot : 128 * (ot + 1), :], in_=outsb[:, ot, :]
            )

    nc.finalize()
    return nc
def _host_prep(x, gn_w, gn_b, qkv_w, qkv_b, proj_w, proj_b, rel_emb):
    """Build the 8 per-core input maps."""
    x = np.asarray(x, dtype=np.float32)
    gn_w = np.asarray(gn_w, dtype=np.float32)
    gn_b = np.asarray(gn_b, dtype=np.float32)
    qkv_w = np.asarray(qkv_w, dtype=np.float32)
    qkv_b = np.asarray(qkv_b, dtype=np.float32)
    proj_w = np.asarray(proj_w, dtype=np.float32)
    proj_b = np.asarray(proj_b, dtype=np.float32)
    rel_emb = np.asarray(rel_emb, dtype=np.float32)

    # relative position bias (matches reference._rel_pos_bias, float32 math)
    dd, hh, ww = np.meshgrid(
        np.arange(D), np.arange(H), np.arange(W), indexing="ij"
    )
    coords = np.stack(
        [dd.ravel(), hh.ravel(), ww.ravel()], axis=-1
    ).astype(np.float32)
    rel = coords[:, None, :] - coords[None, :, :]
    dist = np.sqrt(np.sum(rel * rel, axis=-1, dtype=np.float32)).astype(np.float32)
    buckets = np.clip(
        np.floor(dist / np.float32(MAX_DIST / NUM_BUCKETS)).astype(np.int32),
        0,
        NUM_BUCKETS - 1,
    )
    expb = np.exp(rel_emb[buckets]).astype(np.float32)  # [N, N], symmetric

    import ml_dtypes

    bf16 = ml_dtypes.bfloat16
    projb_eff = (proj_b + proj_w @ qkv_b[2 * C : 3 * C]).astype(np.float32)
    qkvwT = np.ascontiguousarray(qkv_w.T).astype(bf16)
    projwT = np.ascontiguousarray(proj_w.T).astype(bf16)
    gsel = np.zeros((C, GROUPS), np.float32)
    gsel[np.arange(C), np.arange(C) // 64] = 1.0
    gselT = np.ascontiguousarray(gsel.T)
    ones8 = np.ones((128, HEADS), np.float32).astype(bf16)

    xb = x.reshape(B, C, N)
    in_maps = []
    for c in range(NCORES):
        b, qoff = c // 4, (c % 4) * NQ
        xroll = np.roll(xb[b], -qoff, axis=1)
        xc = np.ascontiguousarray(xroll).astype(bf16)
        xres_c = np.ascontiguousarray(xroll[:, 0:NQ])
        bias_c = np.ascontiguousarray(
            np.roll(expb, -qoff, axis=0)[:, qoff : qoff + NQ]
        ).astype(bf16)
        in_maps.append(
            {
                "x": xc,
                "xres": xres_c,
                "qkvwT": qkvwT,
                "projwT": projwT,
                "expbT": bias_c,
                "gnw": gn_w,
                "gnb": gn_b,
                "qkvb": qkv_b,
                "projb": projb_eff,
                "gsel": gsel,
                "gselT": gselT,
                "ones8": ones8,
            }
        )
    return in_maps


def _run(inputs, trace=False, trace_cores=None):
    if "nc" not in _CACHE:
        _CACHE["nc"] = _build()
    nc = _CACHE["nc"]
    in_maps = _host_prep(**inputs)
    last_err = None
    for attempt in range(3):
        try:
            res = run_bass_kernel_spmd(
                nc,
                in_maps,
                core_ids=list(range(NCORES)),
                trace=trace,
                trace_cores=trace_cores,
            )
            break
        except Exception as e:  # transient NRT device errors on first exec
            last_err = e
            import time as _time

            _time.sleep(2.0)
            try:
                import jax

                jax.clear_backends()
            except Exception:
                pass
    else:
        raise last_err
    out = np.empty((B, C, N), np.float32)
    for c in range(NCORES):
        b, qoff = c // 4, (c % 4) * NQ
        out[b][:, qoff : qoff + NQ] = res.results[c]["out"]
    return out.reshape(B, C, D, H, W), res


def kernel(**inputs) -> np.ndarray:
    out, _ = _run(inputs, trace=False)
    return out
